# revision 8
# baseline (speedup 1.0000x reference)
"""W4A4 quantized linear on 8 Trainium2 cores — v2: 4x token x 2x out-feature
sharding (no collectives), LDW-pipelined fp8 SwInterleave matmuls.

Per core: own 1024 tokens (row-reversed), 2048-row weight slice.
  y_blk = dequant_rowwise_quant(x_blk) @ dequant_w4(W_blk)^T + bias_blk

Device algorithm (per core), exact-integer math on the PE:
  setup: unpack W slice to fp8 k-planes (wt_sep[jj, c, nb, i, n]), amax of
         own tokens -> a_scales (+ J-matmul partition flip for the epilogue).
  per rep, per token tile i (8 tiles):
    qb = fp16(x * (1/a_scale) + 1536); q8 = fp8(qb - 1536)   (exact ints)
    qT = byte-pair DMA transpose (SwInterleave stationary format)
    for c in 16: for nb in 4: matmul(ps[:, nb], qT[c], wt_sep[c, nb], SWI)
    y = (ps * a_scale_flip) * wscale + bias  -> DMA out (row-reversed)

PE work: 8 tiles x 64 MMs x 512 cols = 109 us/core; everything else
(~21 MB DMA, ~25 us DVE, ~30 us ACT per rep) hides under it.
"""

import numpy as np
import concourse.bass as bass
import concourse.mybir as mybir
from concourse import bacc
from concourse.tile import TileContext
from concourse.bass_utils import run_bass_kernel_spmd

F8 = mybir.dt.float8e4
F16 = mybir.dt.float16
F32 = mybir.dt.float32
I8 = mybir.dt.int8
I16 = mybir.dt.int16
AOP = mybir.AluOpType
ACTF = mybir.ActivationFunctionType
SWI = mybir.MatmulPerfMode.DoubleRowSwInterleave

N_CORES = 8
TSH = 4            # token shards
NSH = 2            # out-feature shards
M, K, N = 4096, 4096, 4096
MO = M // TSH      # tokens per core (1024)
NS = N // NSH      # out features per core (2048)


def build(repeat=1, x_bufs=6, qt_bufs=5, cast_mod=1, mm_bufs=3,
          mm_only=False):
    T = MO // 128      # token tiles per core (8)
    NT = NS // 128     # weight row blocks (16)
    NB = NS // 512     # 512-col output blocks (4)
    C = K // 256       # contraction chunks (16)
    KP = K // 2        # packed weight columns
    NPB = NT // NB     # 128-row blocks per 512-col block (4)

    nc = bacc.Bacc("TRN2", target_bir_lowering=False, debug=False,
                   num_devices=N_CORES)

    x_d = nc.dram_tensor("x", [MO, K], F16, kind="ExternalInput")  # reversed!
    wp_d = nc.dram_tensor("wp", [NS, KP], I8, kind="ExternalInput")
    ws_d = nc.dram_tensor("wsc", [1, NS], F16, kind="ExternalInput")
    b_d = nc.dram_tensor("bias", [1, NS], F16, kind="ExternalInput")
    y_d = nc.dram_tensor("y", [MO, NS], F16, kind="ExternalOutput")

    with TileContext(nc) as tc:
        with (
            tc.tile_pool(name="const", bufs=1) as cpool,
            tc.tile_pool(name="wsetup", bufs=2) as wpool,
            tc.tile_pool(name="xwork", bufs=2) as xpool,
            tc.tile_pool(name="qtp", bufs=qt_bufs) as qpool,
            tc.tile_pool(name="small", bufs=3) as spool,
            tc.tile_pool(name="epi", bufs=2) as epool,
            tc.tile_pool(name="psum", bufs=mm_bufs, space="PSUM") as ppool,
        ):
            # ---------------- constants ----------------
            wsc_row = cpool.tile([1, NS], F16)
            nc.sync.dma_start(wsc_row[:, :], ws_d.ap())
            wsc_bc = cpool.tile([128, NS], F16)
            nc.gpsimd.partition_broadcast(wsc_bc[:, :], wsc_row[:, :])
            bias_row = cpool.tile([1, NS], F16)
            nc.sync.dma_start(bias_row[:, :], b_d.ap())
            bias_bc = cpool.tile([128, NS], F16)
            nc.gpsimd.partition_broadcast(bias_bc[:, :], bias_row[:, :])
            # anti-diagonal J for the partition flip
            jm = cpool.tile([128, 128], F32)
            nc.vector.memset(jm[:, :], 1.0)
            nc.gpsimd.affine_select(jm[:, :], jm[:, :], pattern=[[1, 128]],
                                    base=-127, channel_multiplier=1,
                                    compare_op=AOP.is_equal, fill=0.0)

            # ---------------- x prefetch (overlap with setup) ----------
            pre_x = {}
            for i in range(min(x_bufs, T)):
                xt = xpool.tile([128, K], F16, tag="x", bufs=x_bufs,
                                name=f"xt_0_{i}")
                nc.sync.dma_start(xt[:, :], x_d[i * 128:(i + 1) * 128, :])
                pre_x[i] = xt

            # ---------------- weight setup ----------------
            # wt_sep[jj, c, nb, i, n]: fp8 W[nb*512+n, 256c+2jj+i], each
            # (c, nb) slice is a contiguous [128, 2, 512] moving operand.
            wt_sep = cpool.tile([128, C, NB, 2, 512], F8)
            for nt in range(NT):
                wp_sb = wpool.tile([128, KP], I8, tag="wp")
                nc.sync.dma_start(wp_sb[:, :],
                                  wp_d[nt * 128:(nt + 1) * 128, :])
                w8 = wpool.tile([128, K], F8, tag="w8")
                w8v = w8[:, :].rearrange("p (j two) -> p j two", two=2)
                # high nibble = floor(b/16) (already sign-extended):
                # fp16(b/16 + 1535.53125) - 1536 via exact magic rounding
                hb = wpool.tile([128, KP], F16, tag="hb")
                nc.scalar.activation(hb[:, :], wp_sb[:, :], ACTF.Copy,
                                     bias=1535.53125, scale=1.0 / 16)
                nc.scalar.activation(w8v[:, :, 1], hb[:, :], ACTF.Copy,
                                     bias=-1536.0, scale=1.0)
                # low nibble: ((b & 15) ^ 8) - 8
                lo4 = wpool.tile([128, KP], I8, tag="lo4")
                nc.vector.tensor_scalar(lo4[:, :], wp_sb[:, :], 15, 8,
                                        op0=AOP.bitwise_and,
                                        op1=AOP.bitwise_xor)
                nc.vector.tensor_scalar(w8v[:, :, 0], lo4[:, :], 8.0, None,
                                        op0=AOP.subtract)
                # byte-pair transpose: [n-row, k] -> [k-pair jj, c, n]
                wt_pairs = wpool.tile([128, C, 128], F16, tag="wtp")
                nc.sync.dma_start_transpose(wt_pairs[:, :, :],
                                            w8[:, :].bitcast(F16))
                wtp8 = wt_pairs[:, :, :].bitcast(F8)  # [128, C, 256]
                nb, off = divmod(nt, NPB)
                sl = slice(off * 128, (off + 1) * 128)
                src = wtp8.rearrange("p c (f two) -> p c two f", two=2)
                # deinterleave planes: i=0 on DVE, i=1 on ACT (balance)
                nc.vector.tensor_copy(wt_sep[:, :, nb, 0, sl], src[:, :, 0, :])
                nc.scalar.copy(wt_sep[:, :, nb, 1, sl], src[:, :, 1, :])

            # ---------------- amax of own tokens ----------------
            # j >= x_bufs reuse the x-tag pool (rotation reclaims the
            # earliest prefetched buffers); rep 0 consumes the surviving
            # last-x_bufs tiles first, in allocation order.
            s_rev = cpool.tile([128, T], F32)
            for j in range(T):
                if j in pre_x:
                    xt = pre_x[j]  # kept alive: main loop rep 0 reuses
                else:
                    xt = xpool.tile([128, K], F16, tag="x", bufs=x_bufs,
                                    name=f"xt_0_{j}")
                    nc.sync.dma_start(xt[:, :], x_d[j * 128:(j + 1) * 128, :])
                    pre_x[j] = xt
                xa = spool.tile([128, K], I16, tag="xa", bufs=1,
                                name=f"xa_{j}")
                nc.vector.tensor_scalar(xa[:, :], xt[:, :].bitcast(I16),
                                        0x7FFF, None, op0=AOP.bitwise_and)
                w = K // 2
                while w >= 512:
                    nc.vector.tensor_tensor(xa[:, :w], xa[:, :w],
                                            xa[:, w:2 * w], op=AOP.max)
                    w //= 2
                mbits = spool.tile([128, 1], I16, tag="mbits")
                nc.vector.tensor_reduce(mbits[:, :], xa[:, :2 * w],
                                        axis=mybir.AxisListType.X,
                                        op=AOP.max)
                nc.vector.tensor_scalar(s_rev[:, j:j + 1],
                                        mbits[:, :].bitcast(F16),
                                        1e-6, 1.0 / 7.0,
                                        op0=AOP.max, op1=AOP.mult)

            # reciprocal for quantization + flipped scales for the epilogue
            sq_all = cpool.tile([128, T], F32)
            nc.vector.reciprocal(sq_all[:, :], s_rev[:, :])
            ps_j = ppool.tile([128, T], F32, tag="psj", bufs=1)
            nc.tensor.matmul(ps_j[:, :], jm[:, :], s_rev[:, :],
                             start=True, stop=True)
            s_flip = cpool.tile([128, T], F32)
            nc.vector.tensor_copy(s_flip[:, :], ps_j[:, :])

            # ---------------- main loop ----------------
            qTc = None
            if mm_only:  # diagnostic: constant stationary, no act pipeline
                qTc = cpool.tile([128, C, 128], F16)
                nc.vector.memset(qTc[:, :, :], 0.251)
            # after amax, only the last x_bufs x-tiles hold valid data
            # (earlier buffers were re-filled by the amax loads); rep 0
            # must consume them first so fresh loads reuse buffers whose
            # pending readers were already emitted (keeps DVE FIFO acyclic)
            stale = T - x_bufs
            for j in range(stale):
                pre_x.pop(j, None)
            order0 = list(range(stale, T)) + list(range(stale))
            for rep in range(repeat):
              for i in (order0 if rep == 0 else range(T)):
                  if mm_only:
                      qT = qTc
                  else:
                      if rep == 0 and i in pre_x:
                          xt = pre_x.pop(i)
                      else:
                          xt = xpool.tile([128, K], F16, tag="x", bufs=x_bufs,
                                          name=f"xtm_{rep}_{i}")
                          nc.sync.dma_start(xt[:, :],
                                            x_d[i * 128:(i + 1) * 128, :])
                      # qb = fp16(x*sq + 1536): exact RNE integer round
                      nc.vector.tensor_scalar(xt[:, :], xt[:, :],
                                              sq_all[:, i:i + 1], 1536.0,
                                              op0=AOP.mult, op1=AOP.add)
                      # q8 = fp8(qb - 1536), alternating ACT/DVE
                      q8 = xpool.tile([128, K], F8, tag="q8",
                                      name=f"q8_{rep}_{i}")
                      if i % cast_mod != cast_mod - 1:
                          nc.scalar.activation(q8[:, :], xt[:, :], ACTF.Copy,
                                               bias=-1536.0, scale=1.0)
                      else:
                          nc.vector.tensor_scalar(q8[:, :], xt[:, :], 1536.0,
                                                  None, op0=AOP.subtract)
                      # pair-transpose: qT[jj, c, f] = (q[f, 256c+2jj], +1)
                      qT = qpool.tile([128, C, 128], F16, tag="qT",
                                      name=f"qT_{rep}_{i}")
                      nc.scalar.dma_start_transpose(qT[:, :, :],
                                                    q8[:, :].bitcast(F16))
                  qT8 = qT[:, :, :].bitcast(F8)  # [128, C, 256]

                  ps0 = ppool.tile([128, 1024], F32, tag="mm",
                                   name=f"ps0_{rep}_{i}")
                  ps1 = ppool.tile([128, 1024], F32, tag="mm",
                                   name=f"ps1_{rep}_{i}")
                  pss = (ps0, ps0, ps1, ps1)
                  for c in range(C):
                      for nb in range(NB):
                          nc.tensor.matmul(
                              pss[nb][:, (nb % 2) * 512:(nb % 2 + 1) * 512],
                              qT8[:, c, :],
                              wt_sep[:, c, nb, :, :],
                              start=(c == 0), stop=(c == C - 1),
                              perf_mode=SWI)
                  # epilogue: y = (ps * a_scale) * wscale + bias
                  t1 = epool.tile([128, NS], F16, tag="t1", name=f"t1_{rep}_{i}")
                  for h, ph in enumerate((ps0, ps1)):
                      nc.vector.scalar_tensor_tensor(
                          t1[:, h * 1024:(h + 1) * 1024], ph[:, :],
                          s_flip[:, i:i + 1],
                          wsc_bc[:, h * 1024:(h + 1) * 1024],
                          op0=AOP.mult, op1=AOP.mult)
                  nc.vector.tensor_tensor(t1[:, :], t1[:, :], bias_bc[:, :],
                                          op=AOP.add)
                  nc.scalar.dma_start(
                      y_d[MO - 128 * (i + 1):MO - 128 * i, :], t1[:, :])

    nc.compile()
    return nc


_CACHE = {}


def _get_nc():
    if "nc" not in _CACHE:
        _CACHE["nc"] = build()
    return _CACHE["nc"]


def _in_maps(x, qweight_packed, w_scales, bias):
    x2 = np.asarray(x).reshape(M, K)
    wsc = np.asarray(w_scales).reshape(N)
    bias = np.asarray(bias).reshape(N)
    in_maps = []
    for core in range(N_CORES):
        ti, ni = divmod(core, NSH)
        xsl = x2[ti * MO:(ti + 1) * MO]
        nsl = slice(ni * NS, (ni + 1) * NS)
        in_maps.append({
            "x": np.ascontiguousarray(xsl[::-1]),
            "wp": np.ascontiguousarray(np.asarray(qweight_packed)[nsl]),
            "wsc": np.ascontiguousarray(wsc[nsl]).reshape(1, NS),
            "bias": np.ascontiguousarray(bias[nsl]).reshape(1, NS),
        })
    return in_maps


def kernel(x, qweight_packed, w_scales, bias):
    nc = _get_nc()
    in_maps = _in_maps(x, qweight_packed, w_scales, bias)
    res = run_bass_kernel_spmd(nc, in_maps, core_ids=list(range(N_CORES)))
    y = np.empty((M, N), np.float16)
    for core in range(N_CORES):
        ti, ni = divmod(core, NSH)
        y[ti * MO:(ti + 1) * MO, ni * NS:(ni + 1) * NS] = res.results[core]["y"]
    return y.reshape(2, 2048, N)
